# revision 10
# baseline (speedup 1.0000x reference)
"""Trainium2 Bass kernel for nn_Attention (LayerNorm + MHA + out-proj).

Sharding: 8 cores = 4 batch elements x 2 query-halves. Each core receives its
batch element's full token sequence (rolled so its 1024 query rows are first),
computes LayerNorm + K/V projections over all 2048 tokens, Q projection and
attention for its 1024 query rows, and the output projection. No collectives.

Layout strategy (single SPMD Bass program, feature-on-partition style):
  x [2048,512] --LN--> xc(bf16) --PE transpose--> xnT [4][128d, 2048tok] bf16
  Kt/Qt per head-pair: [128(2x64), tok] bf16  (W chunk stationary, xnT moving)
  V:                   [128tok, 8x65] bf16    (xnT stationary, W moving;
                                               65th col = ones for row-sums)
  St per (pair,qt,kc): [128k, 2x512q] PSUM    (Kt stationary, Qt moving;
                                               the two heads of a pair run as
                                               concurrent 64-row PE tiles)
  A = exp(St/8)        ACT/DVE -> bf16 SBUF
  O^T per head:        [65, 512q] PSUM accum  (V stationary, A moving)
  normalize: row 64 -> reciprocal_approx_fast -> gpsimd bcast -> DVE mult
  out^T = W_out^T @ O^T (bf16) + b_out; host transposes back.

vs. the previous revision:
  - ln_gamma/ln_beta are folded into W_qkv on the host (W' = diag(gamma) @ W,
    b' = beta @ W); the K/Q evacuations apply b' as a per-partition ACT bias
    and V adds a broadcast row, so the LN apply is a plain (x-mu)*rstd.
  - one PSUM pool for the whole kernel (tags: 2x[128,1024] + 4x[128,512])
    instead of per-phase pools -- no pool-transition barriers.
  - per-token-tile pipeline in phase A (LN -> 4 transposes -> one ACT evac),
    fewer+bigger DMAs, so the PE starts earlier.
  - out-proj for q-tile 0 is emitted in the middle of q-tile 1's attention, so
    its wait on the softmax normalize never blocks the PE queue (the 3.7us
    qt-boundary bubbles previously re-throttled the PE clock for ~13us).
  - LN apply runs on gpsimd, K/Q/y evacuations on ACT, V on DVE to balance.
"""

import numpy as np
import ml_dtypes

import concourse.bass as bass
import concourse.tile as tile
from concourse import bacc, mybir
from concourse.bass_utils import run_bass_kernel_spmd
from concourse.masks import make_identity

F32 = mybir.dt.float32
BF16 = mybir.dt.bfloat16
I16 = mybir.dt.int16
ADD = mybir.AluOpType.add
MULT = mybir.AluOpType.mult

B, N, D = 4, 2048, 512
H, DH = 8, 64
NQ = N // 2          # query rows per core
SCALE = DH ** -0.5   # 0.125
NCORES = 8

# Schraudolph fast-exp on DVE for a subset of key chunks: bf16 bit pattern of
# exp(s*SCALE) ~= int16(round(A*s + B)); softmax numerator/denominator both use
# the same approximate weights, so the ratio error stays small.
SEXP_A = float(SCALE * 128 / np.log(2))
SEXP_B = float(128 * 127 - 4.5)
SEXP_KCS = frozenset((2, 5, 8, 11, 14))  # 5 of 16 chunks go to DVE

QT = NQ // 512       # 2 query tiles of 512
KC = N // 128        # 16 key chunks of 128
TT = N // 128        # 16 token tiles of 128
DC = D // 128        # 4 feature chunks of 128

DEBUG_TAPS = False   # adds DRAM dumps of intermediates (xnT, kt, qt, v)


def build_program(out_dtype=F32):
    nc = bacc.Bacc("TRN2", target_bir_lowering=False, debug=False)

    x_ap = nc.dram_tensor("x", [N, D], F32, kind="ExternalInput").ap()
    wqkv_ap = nc.dram_tensor("w_qkv", [D, 3 * D], BF16, kind="ExternalInput").ap()
    wout_ap = nc.dram_tensor("w_out", [D, D], BF16, kind="ExternalInput").ap()
    bout_ap = nc.dram_tensor("b_out", [D], F32, kind="ExternalInput").ap()
    bkq_ap = nc.dram_tensor("b_kq", [128, 8], F32, kind="ExternalInput").ap()
    bv_ap = nc.dram_tensor("b_v", [1, H * DH], F32, kind="ExternalInput").ap()
    y_ap = nc.dram_tensor("y_t", [D, NQ], BF16, kind="ExternalOutput").ap()
    taps = None
    if DEBUG_TAPS:
        taps = {
            "xnT": nc.dram_tensor("d_xnT", [128, DC, N], BF16, kind="ExternalOutput").ap(),
            "kt": nc.dram_tensor("d_kt", [128, 4, N], BF16, kind="ExternalOutput").ap(),
            "qt": nc.dram_tensor("d_qt", [128, 4, NQ], BF16, kind="ExternalOutput").ap(),
            "v": nc.dram_tensor("d_v", [128, TT, H * (DH + 1)], BF16, kind="ExternalOutput").ap(),
            "ot": nc.dram_tensor("d_ot", [128, 4, NQ], BF16, kind="ExternalOutput").ap(),
        }

    with tile.TileContext(nc) as tc:
        attention_kernel(tc, y_ap, x_ap, wqkv_ap, wout_ap, bout_ap, bkq_ap, bv_ap,
                         taps=taps)
    nc.compile()
    return nc


def attention_kernel(tc, y_ap, x_ap, wqkv_ap, wout_ap, bout_ap, bkq_ap, bv_ap,
                     taps=None):
    nc = tc.nc
    from contextlib import ExitStack

    with ExitStack() as ctx:
        persist = ctx.enter_context(tc.tile_pool(name="persist", bufs=1))
        work = ctx.enter_context(tc.tile_pool(name="work", bufs=3))
        psum = ctx.enter_context(tc.tile_pool(name="psum", bufs=1, space="PSUM"))

        def big_tile(name):
            return psum.tile([128, 1024], F32, tag="big", bufs=2, name=name)

        def sm_tile(name):
            return psum.tile([128, 512], F32, tag="sm", bufs=4, name=name)

        # ---- input DMAs, ordered for the phase-A pipeline: x tile 0 first
        # (smallest possible first-LN latency), weights interleaved so each
        # consumer's data lands before its matmuls reach the PE queue ----
        xts = persist.tile([128, TT, D], F32)
        xsrc = x_ap.rearrange("(g p) d -> p g d", p=128)
        nc.sync.dma_start(xts[:, 0, :], xsrc[:, 0, :])
        nc.sync.dma_start(xts[:, 1, :], xsrc[:, 1, :])
        nc.sync.dma_start(xts[:, 2:4, :], xsrc[:, 2:4, :])

        wq_sb = persist.tile([128, DC, 3 * D], BF16)
        wsrc = wqkv_ap.rearrange("(c p) e -> p c e", p=128)
        nc.sync.dma_start(wq_sb[:, :, D:2 * D], wsrc[:, :, D:2 * D])   # K
        nc.sync.dma_start(xts[:, 4:8, :], xsrc[:, 4:8, :])
        nc.sync.dma_start(wq_sb[:, :, 2 * D:3 * D], wsrc[:, :, 2 * D:3 * D])  # V
        nc.sync.dma_start(wq_sb[:, :, 0:D], wsrc[:, :, 0:D])           # Q
        nc.sync.dma_start(xts[:, 8:12, :], xsrc[:, 8:12, :])
        nc.sync.dma_start(xts[:, 12:16, :], xsrc[:, 12:16, :])

        bkq_sb = persist.tile([128, 8], F32)
        nc.sync.dma_start(bkq_sb, bkq_ap)
        bias_sb = persist.tile([128, DC], F32)
        nc.sync.dma_start(bias_sb, bout_ap.rearrange("(c p) -> p c", p=128))
        bv_row = persist.tile([1, H * DH], F32)
        nc.sync.dma_start(bv_row, bv_ap)
        wo_sb = persist.tile([128, DC, D], BF16)
        nc.sync.dma_start(wo_sb, wout_ap.rearrange("(c p) e -> p c e", p=128))

        # ---- constants ----
        identity = persist.tile([128, 128], BF16)
        make_identity(nc, identity)
        eps_sb = persist.tile([128, 1], F32)
        nc.vector.memset(eps_sb, 1e-5)
        bv_sb = persist.tile([128, H * DH], F32)
        nc.gpsimd.partition_broadcast(bv_sb, bv_row)

        # ---- persistent activations ----
        xnT = persist.tile([128, DC, N], BF16)        # [d-part, dchunk, tok]
        kt_all = persist.tile([128, 4, N], BF16)      # [2x64 head rows, pair, tok]
        qt_all = persist.tile([128, 4, NQ], BF16)     # [2x64 head rows, pair, qtok]
        v_all = persist.tile([128, TT, H * (DH + 1)], BF16)  # [tok, tt, 8x65]
        ot_all = persist.tile([128, 4, NQ], BF16)     # [2x64 inner rows, pair, qtok]

        nc.gpsimd.memset(v_all, 1.0)

        # ---- Phase A: per-token-tile pipeline (LN -> transpose -> evac),
        # V per tile (lagged 2), K/Q per 1024-token range ----
        def v_proj(tt):
            pv = sm_tile("pv")
            for dc in range(DC):
                nc.tensor.matmul(
                    pv,
                    lhsT=xnT[:, dc, tt * 128:(tt + 1) * 128],
                    rhs=wq_sb[:, dc, 2 * D:3 * D],
                    start=(dc == 0), stop=(dc == DC - 1),
                )
            nc.vector.tensor_tensor(
                v_all[:, tt, :].rearrange("p (h e) -> p h e", e=DH + 1)[:, :, 0:DH],
                pv.rearrange("p (h d) -> p h d", d=DH),
                bv_sb.rearrange("p (h d) -> p h d", d=DH),
                ADD,
            )

        def kq_proj(sec, base, rlo, dst, dst_lo, bias_col):
            # sec: column offset in wq_sb (K or Q section); 2 ranges of 512
            # tokens accumulated into one [128,1024] window, single ACT evac
            for p in range(4):
                pk = big_tile("pk")
                for r in range(2):
                    for dc in range(DC):
                        nc.tensor.matmul(
                            pk[:, r * 512:(r + 1) * 512],
                            lhsT=wq_sb[:, dc, sec + p * 128:sec + (p + 1) * 128],
                            rhs=xnT[:, dc, (rlo + r) * 512:(rlo + r + 1) * 512],
                            start=(dc == 0), stop=(dc == DC - 1),
                        )
                nc.scalar.activation(
                    out=dst[:, p, dst_lo:dst_lo + 1024], in_=pk,
                    func=mybir.ActivationFunctionType.Identity,
                    bias=bkq_sb[:, bias_col + p:bias_col + p + 1], scale=1.0,
                )

        for tt in range(TT):
            xt = xts[:, tt, :]
            stats = work.tile([128, 6], F32, tag="stats", bufs=6, name="stats")
            nc.vector.bn_stats(out=stats, in_=xt)
            mv = work.tile([128, 2], F32, tag="mv", bufs=6, name="mv")
            nc.vector.bn_aggr(out=mv, in_=stats)
            rstd = work.tile([128, 1], F32, tag="rstd", bufs=6, name="rstd")
            nc.scalar.activation(
                out=rstd, in_=mv[:, 1:2],
                func=mybir.ActivationFunctionType.Sqrt,
                bias=eps_sb, scale=1.0,
            )
            nc.vector.reciprocal(out=rstd, in_=rstd)
            xc = work.tile([128, D], BF16, tag="xc", bufs=6, name="xc")
            nc.vector.tensor_scalar(
                out=xc, in0=xt, scalar1=mv[:, 0:1], scalar2=rstd,
                op0=mybir.AluOpType.subtract, op1=MULT,
            )
            tpg = psum.tile([128, 512], BF16, tag="sm", bufs=4, name="tpg")
            for dc in range(DC):
                nc.tensor.transpose(
                    tpg[:, dc * 128:(dc + 1) * 128],
                    xc[:, dc * 128:(dc + 1) * 128], identity,
                )
            nc.scalar.activation(
                out=xnT[:, :, tt * 128:(tt + 1) * 128],
                in_=tpg.rearrange("p (c e) -> p c e", e=128),
                func=mybir.ActivationFunctionType.Identity, scale=1.0,
            )
            if tt >= 2:
                v_proj(tt - 2)
            if tt == 7:
                kq_proj(D, 0, 0, kt_all, 0, 0)       # K ranges 0-1
                kq_proj(0, 0, 0, qt_all, 0, 4)       # Q (all 1024 query rows)
        v_proj(TT - 2)
        v_proj(TT - 1)
        kq_proj(D, 0, 2, kt_all, 1024, 0)            # K ranges 2-3

        if taps is not None:
            nc.sync.dma_start(taps["xnT"], xnT)
            nc.sync.dma_start(taps["kt"], kt_all)
            nc.sync.dma_start(taps["qt"], qt_all)
            nc.sync.dma_start(taps["v"], v_all)

        # ---- Phase C: attention; out-proj for qt is emitted inside qt+1's
        # p-loop so its normalize-wait never blocks the PE queue ----
        def out_proj(qt):
            yf = work.tile([128, DC, 512], BF16, tag="yf", bufs=2, name="yf")
            for dm in range(DC):
                yp = sm_tile("yp")
                for p in range(4):
                    nc.tensor.matmul(
                        yp,
                        lhsT=wo_sb[:, p, dm * 128:(dm + 1) * 128],
                        rhs=ot_all[:, p, qt * 512:(qt + 1) * 512],
                        start=(p == 0), stop=(p == 3),
                    )
                nc.scalar.activation(
                    out=yf[:, dm, :], in_=yp,
                    func=mybir.ActivationFunctionType.Identity,
                    bias=bias_sb[:, dm:dm + 1], scale=1.0,
                )
            nc.sync.dma_start(
                y_ap.rearrange("(c p) q -> p c q", p=128)[:, :, qt * 512:(qt + 1) * 512],
                yf,
            )

        for qt in range(QT):
            for p in range(4):
                oacc = [sm_tile(f"o{i}") for i in range(2)]
                ats = {}
                LAG = 2
                for kc in range(KC + LAG):
                    if kc < KC:
                        st = big_tile("st")
                        for half in range(2):
                            nc.tensor.matmul(
                                st[:, half * 512:(half + 1) * 512],
                                lhsT=kt_all[64 * half:64 * half + 64, p,
                                            kc * 128:(kc + 1) * 128],
                                rhs=qt_all[64 * half:64 * half + 64, p,
                                           qt * 512:(qt + 1) * 512],
                                start=True, stop=True,
                            )
                    if kc >= LAG:
                        pkc = kc - LAG
                        pat = ats.pop(pkc)
                        for half in range(2):
                            h = 2 * p + half
                            nc.tensor.matmul(
                                oacc[half][0:DH + 1, :],
                                lhsT=v_all[:, pkc, h * (DH + 1):(h + 1) * (DH + 1)],
                                rhs=pat[:, half * 512:(half + 1) * 512],
                                start=(pkc == 0), stop=(pkc == KC - 1),
                            )
                    if kc < KC:
                        at = work.tile([128, 1024], BF16, tag="at", bufs=6, name="at")
                        if kc in SEXP_KCS:
                            nc.vector.tensor_scalar(
                                out=at.bitcast(I16), in0=st,
                                scalar1=SEXP_A, scalar2=SEXP_B,
                                op0=MULT, op1=ADD,
                            )
                        else:
                            nc.scalar.activation(
                                out=at, in_=st,
                                func=mybir.ActivationFunctionType.Exp, scale=SCALE,
                            )
                        ats[kc] = at
                for half in range(2):
                    o_acc = oacc[half]
                    s_sb = work.tile([1, 512], F32, tag="s_sb", bufs=4, name="s_sb")
                    nc.vector.tensor_copy(s_sb, o_acc[DH:DH + 1, :])
                    r_sb = work.tile([1, 512], F32, tag="r_sb", bufs=4, name="r_sb")
                    nc.vector.reciprocal_approx_fast(out=r_sb, in_=s_sb)
                    cb_sb = work.tile([DH, 512], F32, tag="cb", bufs=4, name="cb")
                    nc.gpsimd.partition_broadcast(cb_sb, r_sb)
                    nc.vector.tensor_tensor(
                        ot_all[64 * half:64 * half + 64, p,
                               qt * 512:(qt + 1) * 512],
                        o_acc[0:DH, :], cb_sb, MULT,
                    )
                if qt == 1 and p == 0:
                    out_proj(0)
        out_proj(1)
        if taps is not None:
            nc.sync.dma_start(taps["ot"], ot_all)


_CACHED_NC = None


def _get_program():
    global _CACHED_NC
    if _CACHED_NC is None:
        _CACHED_NC = build_program()
    return _CACHED_NC


def make_in_maps(x, ln_gamma, ln_beta, W_qkv, W_out, b_out):
    x = np.asarray(x, dtype=np.float32)
    gamma = np.asarray(ln_gamma, dtype=np.float32)
    beta = np.asarray(ln_beta, dtype=np.float32)
    wqkv_f = np.asarray(W_qkv, dtype=np.float32)
    # fold LN gamma/beta into the QKV projection: (z*g+b)@W = z@(g[:,None]*W) + b@W
    wqkv_folded = (gamma[:, None] * wqkv_f).astype(ml_dtypes.bfloat16)
    b_qkv = beta @ wqkv_f  # [3*D] f32
    b_kq = np.empty((128, 8), dtype=np.float32)
    for p in range(4):
        b_kq[:, p] = b_qkv[D + p * 128:D + (p + 1) * 128]      # K pairs
        b_kq[:, 4 + p] = b_qkv[p * 128:(p + 1) * 128]          # Q pairs
    b_v = np.ascontiguousarray(b_qkv[2 * D:3 * D][None, :])    # [1, 512]

    wout_bf = np.asarray(W_out, dtype=np.float32).astype(ml_dtypes.bfloat16)
    bout = np.asarray(b_out, dtype=np.float32)
    in_maps = []
    for c in range(NCORES):
        b, qh = c // 2, c % 2
        xb = np.roll(x[b], -NQ * qh, axis=0)  # query rows first
        in_maps.append({
            "x": np.ascontiguousarray(xb),
            "w_qkv": wqkv_folded,
            "w_out": wout_bf,
            "b_out": bout,
            "b_kq": b_kq,
            "b_v": b_v,
        })
    return in_maps


def kernel(x, ln_gamma, ln_beta, W_qkv, W_out, b_out):
    nc = _get_program()
    in_maps = make_in_maps(x, ln_gamma, ln_beta, W_qkv, W_out, b_out)
    res = run_bass_kernel_spmd(nc, in_maps, core_ids=list(range(NCORES)))

    y = np.empty((B, N, D), dtype=np.float32)
    for c in range(NCORES):
        b, qh = c // 2, c % 2
        y[b, NQ * qh:NQ * (qh + 1), :] = res.results[c]["y_t"].astype(np.float32).T
    return y
